# revision 1
# baseline (speedup 1.0000x reference)
"""Trainium2 Bass kernel for nn_MinimalAttention (B=1, S=4096, d_model=768,
H=12, Dh=64, post-softmax causal mask).

Sharding (8 cores): 4 head-groups (3 heads each) x 2 sequence shards.
Each seq shard owns 2048 query rows as 16 128-row subtiles, mod-4
interleaved across the sequence so the causal-mask work pattern is
identical on every core (the SPMD program is branch-free; all per-core
variation is input data: weight slices, pre-gathered xTq columns, mask
tiles).

Per core:
  K^T/Q^T projections in float32r (full-rate fp32 storage), V projection
  in bf16; scores^T = K_h^T slice x Q_h^T per 128-key tile (bf16, PSUM
  f32); exp on ScalarE writing bf16 E tiles; y accumulation as
  y[q,0:64] += E_kt^T @ V_kt with a fused ones column (y[q,64] becomes
  the full softmax denominator; diagonal-window tiles use host-provided
  M / 1-M masks so numerator masking and denominator completeness both
  hold); per-partition reciprocal + scale; PE transpose to y^T; partial
  output projection outT = W_out_slice^T chunk @ y^T.

Host sums the 4 head-group partials per shard, adds b_out, and scatters
the interleaved rows back.
"""
import sys

sys.path.insert(0, "/opt/trn_rl_repo")

import numpy as np
import ml_dtypes

S, D, H, DH = 4096, 768, 12, 64
N_CORES = 8
GD = 192          # head-group dims (3 heads)
LT = 16           # local 128-row subtiles per core (2048 q rows)
NK = 32           # key tiles

_cache = {}


def _g_of(s, t):
    k = t // 2
    if s == 0:
        return 4 * k + (0 if t % 2 == 0 else 3)
    return 4 * k + (1 if t % 2 == 0 else 2)


def _qcols(s):
    idx = []
    for t in range(LT):
        g = _g_of(s, t)
        idx.extend(range(g * 128, g * 128 + 128))
    return np.array(idx)


def _masks(s):
    M = np.zeros((8, 128, 128), np.float32)
    for p in (0, 1):
        delta = _g_of(s, p)  # k4 == 0 for t in (0,1)
        for j in range(4):
            if j < delta:
                M[p * 4 + j] = 1.0
            elif j == delta:
                M[p * 4 + j] = (
                    np.arange(128)[:, None] <= np.arange(128)[None, :]
                ).astype(np.float32)
    return M, 1.0 - M


def _build_program(reps=1, parts="all"):
    import concourse.bass as bass
    import concourse.mybir as mybir
    import concourse.tile as tile
    from concourse import bacc

    f32 = mybir.dt.float32
    f32r = mybir.dt.float32r
    bf16 = mybir.dt.bfloat16
    Exp = mybir.ActivationFunctionType.Exp
    Ident = mybir.ActivationFunctionType.Identity
    mult = mybir.AluOpType.mult
    add = mybir.AluOpType.add

    nc = bacc.Bacc(
        "TRN2",
        target_bir_lowering=False,
        debug=False,
        enable_asserts=False,
        num_devices=N_CORES,
    )

    d_xT = nc.dram_tensor("xt_in", [D, S], bf16, kind="ExternalInput").ap()
    d_xTq = nc.dram_tensor("xtq_in", [D, 2048], bf16, kind="ExternalInput").ap()
    d_wq = nc.dram_tensor("wq_in", [D, GD], bf16, kind="ExternalInput").ap()
    d_wk = nc.dram_tensor("wk_in", [D, GD], bf16, kind="ExternalInput").ap()
    d_wv = nc.dram_tensor("wv_in", [D, GD], bf16, kind="ExternalInput").ap()
    d_bq = nc.dram_tensor("bq_in", [GD, 1], f32, kind="ExternalInput").ap()
    d_bk = nc.dram_tensor("bk_in", [GD, 1], f32, kind="ExternalInput").ap()
    d_bvb = nc.dram_tensor("bvb_in", [128, GD], f32, kind="ExternalInput").ap()
    d_wo = nc.dram_tensor("wo_in", [GD, D], bf16, kind="ExternalInput").ap()
    d_mm = nc.dram_tensor("mm_in", [8, 128, 128], bf16, kind="ExternalInput").ap()
    d_mc = nc.dram_tensor("mc_in", [8, 128, 128], bf16, kind="ExternalInput").ap()
    d_id = nc.dram_tensor("id_in", [128, 128], bf16, kind="ExternalInput").ap()
    d_out = nc.dram_tensor("outt_out", [D, 2048], f32, kind="ExternalOutput").ap()

    def r(ap):
        return ap.bitcast(f32r)

    with tile.TileContext(nc) as tc:
        xTr = d_xT.rearrange("(k p) n -> p k n", p=128)     # [128, 6, 4096]
        xTqr = d_xTq.rearrange("(k p) n -> p k n", p=128)   # [128, 6, 2048]
        wqr = d_wq.rearrange("(k p) m -> p k m", p=128)
        wkr = d_wk.rearrange("(k p) m -> p k m", p=128)
        wvr = d_wv.rearrange("(k p) m -> p k m", p=128)
        mmr = d_mm.rearrange("m p n -> p m n")
        mcr = d_mc.rearrange("m p n -> p m n")
        outr = d_out.rearrange("(m p) q -> p m q", p=128)   # [128, 6, 2048]
        with tc.tile_pool(name="const", bufs=1) as cp:
            wq_sb = cp.tile([128, 6, GD], bf16, tag="wq")
            wk_sb = cp.tile([128, 6, GD], bf16, tag="wk")
            wv_sb = cp.tile([128, 6, GD], bf16, tag="wv")
            wo0 = cp.tile([128, D], bf16, tag="wo0")
            wo1 = cp.tile([64, D], bf16, tag="wo1")
            bq0 = cp.tile([128, 1], f32, tag="bq0")
            bq1 = cp.tile([64, 1], f32, tag="bq1")
            bk0 = cp.tile([128, 1], f32, tag="bk0")
            bk1 = cp.tile([64, 1], f32, tag="bk1")
            bvb = cp.tile([128, GD], f32, tag="bvb")
            mm_sb = cp.tile([128, 8, 128], bf16, tag="mm")
            mc_sb = cp.tile([128, 8, 128], bf16, tag="mc")
            id_sb = cp.tile([128, 128], bf16, tag="ident")
            ones = cp.tile([128, 1], bf16, tag="ones")
            KT0 = cp.tile([128, S], bf16, tag="KT0")
            KT1 = cp.tile([64, S], bf16, tag="KT1")
            QT0 = cp.tile([128, 2048], bf16, tag="QT0")
            QT1 = cp.tile([64, 2048], bf16, tag="QT1")
            Vb = cp.tile([128, 3, NK, 65], bf16, tag="Vb")
            yT0 = cp.tile([128, 2048], bf16, tag="yT0")
            yT1 = cp.tile([64, 2048], bf16, tag="yT1")

            nc.sync.dma_start(out=wq_sb[:], in_=wqr[:])
            nc.sync.dma_start(out=wk_sb[:], in_=wkr[:])
            nc.sync.dma_start(out=wv_sb[:], in_=wvr[:])
            nc.sync.dma_start(out=wo0[:], in_=d_wo[0:128, :])
            nc.sync.dma_start(out=wo1[:], in_=d_wo[128:GD, :])
            nc.sync.dma_start(out=bq0[:], in_=d_bq[0:128, :])
            nc.sync.dma_start(out=bq1[:], in_=d_bq[128:GD, :])
            nc.sync.dma_start(out=bk0[:], in_=d_bk[0:128, :])
            nc.sync.dma_start(out=bk1[:], in_=d_bk[128:GD, :])
            nc.sync.dma_start(out=bvb[:], in_=d_bvb[:, :])
            nc.sync.dma_start(out=mm_sb[:], in_=mmr[:])
            nc.sync.dma_start(out=mc_sb[:], in_=mcr[:])
            nc.sync.dma_start(out=id_sb[:], in_=d_id[:, :])
            nc.vector.memset(ones[:], 1.0)
            nc.vector.memset(Vb[:, :, :, 64:65], 1.0)

            import contextlib
            loop_ctx = tc.For_i(0, reps, 1) if reps > 1 else contextlib.nullcontext()
            with (
                tc.tile_pool(name="xsl", bufs=4) as xp,
                tc.tile_pool(name="psA", bufs=1, space="PSUM") as pa,
                tc.tile_pool(name="psB", bufs=1, space="PSUM") as pb,
                tc.tile_pool(name="psY", bufs=1, space="PSUM") as py,
                tc.tile_pool(name="psT", bufs=1, space="PSUM") as pt,
                tc.tile_pool(name="epool", bufs=3) as ep,
                tc.tile_pool(name="small", bufs=3) as sp,
                tc.tile_pool(name="ocp", bufs=3) as op_,
                loop_ctx,
            ):
                def qproj(qb):
                    xq = xp.tile([128, 6, 512], bf16, tag="xq", name=f"xq{qb}")
                    nc.sync.dma_start(
                        out=xq[:], in_=xTqr[:, :, qb * 512:(qb + 1) * 512]
                    )
                    for i, (msz, off, QT_t, bq_t) in enumerate(
                        ((128, 0, QT0, bq0), (64, 128, QT1, bq1))
                    ):
                        pool, tag = (py, "psY") if i == 0 else (pt, "psT")
                        ps = pool.tile([msz, 512], f32, tag=tag, name=f"qps{qb}_{i}")
                        for k in range(6):
                            nc.tensor.matmul(
                                ps[:],
                                wq_sb[:, k, off:off + msz],
                                xq[:, k, :],
                                start=(k == 0),
                                stop=(k == 5),
                            )
                        nc.vector.tensor_scalar_add(
                            QT_t[:, qb * 512:(qb + 1) * 512], ps[:], bq_t[:]
                        )

                qproj(0)

                # ---- K^T (+bk); xs slabs stay alive for the V pass ----
                xs_slabs = []
                for nb in range(8):
                    xs = xp.tile([128, 6, 512], bf16, tag="xq", name=f"xs{nb}")
                    nc.sync.dma_start(
                        out=xs[:], in_=xTr[:, :, nb * 512:(nb + 1) * 512]
                    )
                    xs_slabs.append(xs)
                    for i, (msz, off, KT_t, bk_t) in enumerate(
                        ((128, 0, KT0, bk0), (64, 128, KT1, bk1))
                    ):
                        pool, tag = (py, "psY") if i == 0 else (pt, "psT")
                        ps = pool.tile([msz, 512], f32, tag=tag, name=f"kps{nb}_{i}")
                        for k in range(6):
                            nc.tensor.matmul(
                                ps[:],
                                wk_sb[:, k, off:off + msz],
                                xs[:, k, :],
                                start=(k == 0),
                                stop=(k == 5),
                            )
                        nc.vector.tensor_scalar_add(
                            KT_t[:, nb * 512:(nb + 1) * 512], ps[:], bk_t[:]
                        )
                for qb in range(1, 4):
                    qproj(qb)

                # V projection: own DMAs + low priority -> fills PE gaps
                # under the ACT-bound attention phase. Needed only by phase 2.
                for nb in range(8):
                    xv = xp.tile([128, 6, 512], bf16, tag="xv", name=f"xv{nb}", bufs=2)
                    nc.sync.dma_start(
                        out=xv[:], in_=xTr[:, :, nb * 512:(nb + 1) * 512]
                    )
                    for ms in range(4):
                        kt = nb * 4 + ms
                        ps = pt.tile([128, GD], f32, tag="psT", name=f"vps{nb}_{ms}")
                        for k in range(6):
                            nc.tensor.matmul(
                                ps[:],
                                xv[:, k, ms * 128:(ms + 1) * 128],
                                wv_sb[:, k, :],
                                start=(k == 0),
                                stop=(k == 5),
                            )
                        nc.vector.tensor_tensor(
                            Vb[:, :, kt, 0:64],
                            ps[:].rearrange("p (h d) -> p h d", h=3),
                            bvb[:].rearrange("p (h d) -> p h d", h=3),
                            add,
                        )


                # ---------------- attention (qb outer, h inner) -------------
                groups = [
                    (0, 4), (4, 2), (6, 4), (10, 2), (12, 4), (16, 2),
                    (18, 4), (22, 2), (24, 4), (28, 2), (30, 2),
                ]
                for qb in range(4):
                    for h in range(3):
                        if h < 2:
                            KTh = KT0[64 * h:64 * (h + 1), :]
                            QTh = QT0[64 * h:64 * (h + 1), :]
                        else:
                            KTh = KT1[0:64, :]
                            QTh = QT1[0:64, :]
                        E = ep.tile([128, NK, 512], bf16, tag="E", name=f"E{qb}_{h}")
                        import contextlib
                        prio_ctx = tc.high_priority() if qb == 0 else contextlib.nullcontext()
                        with prio_ctx:
                            for k0, gsz in groups:
                                pool = pa if gsz == 4 else pb
                                tag = "psA" if gsz == 4 else "psB"
                                ps = pool.tile(
                                    [128, gsz * 512], f32, tag=tag, name=f"sc{qb}_{h}_{k0}"
                                )
                                for i in range(gsz):
                                    kt = k0 + i
                                    nc.tensor.matmul(
                                        ps[:, i * 512:(i + 1) * 512],
                                        KTh[:, kt * 128:(kt + 1) * 128],
                                        QTh[:, qb * 512:(qb + 1) * 512],
                                        start=True,
                                        stop=True,
                                    )
                                nc.scalar.activation(
                                    E[:, k0:k0 + gsz, :], ps[:, 0:gsz * 512], Exp
                                )
                        if parts == "ph1":
                            if h < 2:
                                nc.vector.tensor_copy(yT0[64 * h:64 * (h + 1), qb * 512:qb * 512 + 64], E[0:64, 0, 0:64])
                            else:
                                nc.vector.tensor_copy(yT1[0:64, qb * 512:qb * 512 + 64], E[0:64, 0, 0:64])
                            continue
                        # pre-compute all masked tiles for this unit on DVE
                        em_all = {}
                        ec_all = {}
                        for tl_i in range(4):
                            t = qb * 4 + tl_i
                            k4 = 4 * (t // 2)
                            p = t % 2
                            tsl = slice(tl_i * 128, (tl_i + 1) * 128)
                            for j in range(4):
                                kt = k4 + j
                                em = sp.tile([128, 128], bf16, tag="em", name=f"em{t}_{h}_{j}", bufs=18)
                                nc.vector.tensor_tensor(
                                    em[:], E[:, kt, tsl], mm_sb[:, p * 4 + j, :], mult
                                )
                                em_all[(tl_i, j)] = em
                                ec = sp.tile([128, 128], bf16, tag="ec", name=f"ec{t}_{h}_{j}", bufs=18)
                                nc.vector.tensor_tensor(
                                    ec[:], E[:, kt, tsl], mc_sb[:, p * 4 + j, :], mult
                                )
                                ec_all[(tl_i, j)] = ec
                        for tl_i in range(4):
                            t = qb * 4 + tl_i
                            k4 = 4 * (t // 2)
                            p = t % 2
                            tsl = slice(tl_i * 128, (tl_i + 1) * 128)
                            yp = py.tile([128, 65], f32, tag="psY", name=f"yp{t}_{h}")
                            first = True
                            for kt in range(k4):
                                nc.tensor.matmul(
                                    yp[:],
                                    E[:, kt, tsl],
                                    Vb[:, h, kt, :],
                                    start=first,
                                    stop=False,
                                )
                                first = False
                            for j in range(4):
                                kt = k4 + j
                                nc.tensor.matmul(
                                    yp[:], em_all[(tl_i, j)][:], Vb[:, h, kt, :],
                                    start=first, stop=False,
                                )
                                first = False
                                nc.tensor.matmul(
                                    yp[:, 64:65], ec_all[(tl_i, j)][:], ones[:],
                                    start=False, stop=False,
                                )
                            for kt in range(k4 + 4, NK):
                                nc.tensor.matmul(
                                    yp[:, 64:65],
                                    E[:, kt, tsl],
                                    ones[:],
                                    start=False,
                                    stop=(kt == NK - 1),
                                )
                            rc = sp.tile([128, 1], f32, tag="rc", name=f"rc{t}_{h}")
                            nc.vector.reciprocal(rc[:], yp[:, 64:65])
                            ysb = sp.tile([128, 64], bf16, tag="ysb", name=f"ysb{t}_{h}")
                            nc.vector.tensor_scalar_mul(ysb[:], yp[:, 0:64], rc[:])
                            tp = pt.tile([64, 128], bf16, tag="psT", name=f"tp{t}_{h}")
                            nc.tensor.transpose(tp[:], ysb[:], id_sb[:])
                            if h < 2:
                                dst = yT0[64 * h:64 * (h + 1), t * 128:(t + 1) * 128]
                            else:
                                dst = yT1[0:64, t * 128:(t + 1) * 128]
                            nc.vector.tensor_copy(dst, tp[:])

                    # ---- output projection for this qb ----
                    for mt in range(6):
                        ps = pt.tile([128, 512], f32, tag="psT", name=f"ops{qb}_{mt}")
                        nc.tensor.matmul(
                            ps[:],
                            wo0[:, mt * 128:(mt + 1) * 128],
                            yT0[:, qb * 512:(qb + 1) * 512],
                            start=True,
                            stop=False,
                        )
                        nc.tensor.matmul(
                            ps[:],
                            wo1[:, mt * 128:(mt + 1) * 128],
                            yT1[:, qb * 512:(qb + 1) * 512],
                            start=False,
                            stop=True,
                        )
                        oc = op_.tile([128, 512], f32, tag="ocp", name=f"oc{qb}_{mt}")
                        nc.vector.tensor_copy(oc[:], ps[:])
                        nc.sync.dma_start(
                            out=d_out[mt * 128:(mt + 1) * 128, qb * 512:(qb + 1) * 512],
                            in_=oc[:],
                        )

    nc.compile()
    return nc


def _get_program(reps=1, parts="all"):
    key = ("nc", reps, parts)
    if key not in _cache:
        _cache[key] = _build_program(reps, parts)
    return _cache[key]


def shard_inputs(x, W_qkv, b_qkv, W_out, b_out):
    """Build the 8 per-core input maps."""
    bf = ml_dtypes.bfloat16
    xT = np.ascontiguousarray(x[0].T.astype(np.float32)).astype(bf)  # [D, S]
    ident = np.eye(128, dtype=np.float32).astype(bf)
    in_maps = []
    per_s = {}
    for s in (0, 1):
        cols = _qcols(s)
        M, Mc = _masks(s)
        per_s[s] = (
            np.ascontiguousarray(xT[:, cols]),
            M.astype(bf),
            Mc.astype(bf),
        )
    for c in range(N_CORES):
        hg, s = c // 2, c % 2
        hsl = slice(GD * hg, GD * (hg + 1))
        xTq, M, Mc = per_s[s]
        wq = np.ascontiguousarray((W_qkv[0:768][hsl] / 8.0).T.astype(np.float32)).astype(bf)
        wk = np.ascontiguousarray(W_qkv[768:1536][hsl].T.astype(np.float32)).astype(bf)
        wv = np.ascontiguousarray(W_qkv[1536:2304][hsl].T.astype(np.float32)).astype(bf)
        bq = (b_qkv[0:768][hsl] / 8.0).astype(np.float32).reshape(GD, 1)
        bk = b_qkv[768:1536][hsl].astype(np.float32).reshape(GD, 1)
        bv = b_qkv[1536:2304][hsl].astype(np.float32)
        bvb = np.ascontiguousarray(np.broadcast_to(bv[None, :], (128, GD)))
        wo = np.ascontiguousarray(W_out[:, hsl].T.astype(np.float32)).astype(bf)
        in_maps.append(
            {
                "xt_in": xT,
                "xtq_in": xTq,
                "wq_in": wq,
                "wk_in": wk,
                "wv_in": wv,
                "bq_in": bq,
                "bk_in": bk,
                "bvb_in": bvb,
                "wo_in": wo,
                "mm_in": M,
                "mc_in": Mc,
                "id_in": ident,
            }
        )
    return in_maps


def gather_output(results, b_out):
    out = np.zeros((S, D), np.float32)
    for s in (0, 1):
        acc = np.zeros((2048, D), np.float32)
        for hg in range(4):
            c = hg * 2 + s
            acc += results[c]["outt_out"].T
        out[_qcols(s)] = acc + b_out[None, :].astype(np.float32)
    return out.reshape(1, S, D)


def kernel(x, W_qkv, b_qkv, W_out, b_out):
    from concourse.bass_utils import run_bass_kernel_spmd

    x = np.asarray(x)
    W_qkv = np.asarray(W_qkv)
    b_qkv = np.asarray(b_qkv)
    W_out = np.asarray(W_out)
    b_out = np.asarray(b_out)
    nc = _get_program()
    in_maps = shard_inputs(x, W_qkv, b_qkv, W_out, b_out)
    res = run_bass_kernel_spmd(nc, in_maps, list(range(N_CORES)))
    return gather_output(res.results, b_out)



# revision 23
# speedup vs baseline: 1.0408x; 1.0408x over previous
"""Trainium2 Bass kernel for nn_MinimalAttention (B=1, S=4096, d_model=768,
H=12, Dh=64, post-softmax causal mask).

Sharding (8 cores): 4 head-groups (3 heads) x 2 sequence shards.
Seq shard s owns the 2048 q rows with tile-index g === s (mod 2): local
q-subtile t (16 per core, 128 rows) maps to global q-tile g = 2t + s, so
the causal structure (2t full key-tiles, 2 masked band tiles, rest
strictly-upper) is identical on every core; all per-core variation is
input data (weight slices, gathered xTq columns, 2 mask tiles).

Engine split per core:
  PE     : K/Q/V projections, scores^T = K_h^T x Q_h^T per 128-key tile
           (bf16, PSUM f32), numerator y[q,0:65] += E^T @ [V|1] with the
           fused ones column accumulating the denominator, ~free 1-column
           ones-matmuls adding the upper-region denominator, packed
           transposes, output projection.
  ACT    : exact exp for the causal lower + diagonal band region.
  DVE    : Schraudolph bit-trick exp (f32 -> int16 code, bitcast bf16,
           ~+-3% sawtooth, zero-mean tuned) for the strictly-upper region
           which only feeds the softmax denominator (error averages out
           over ~2k keys), diagonal masking (em), normalize, psum->SBUF
           copies with fused bias.

Emission is software-pipelined: the post-exp work of unit i-1 (numerator
chains, normalize, transposes) and the phase-A tail (K slabs 4-7, Q 1-3,
V projection) are interleaved between the score groups of unit i so the
in-order PE queue never parks on a not-yet-exp'd psum.

Host sums the 4 head-group partials per shard, adds b_out, and scatters
the mod-2 interleaved rows back.
"""
import sys

sys.path.insert(0, "/opt/trn_rl_repo")

import numpy as np
import ml_dtypes

S, D, H, DH = 4096, 768, 12, 64
N_CORES = 8
GD = 192          # head-group dims (3 heads)
LT = 16           # local 128-row subtiles per core (2048 q rows)
NK = 32           # key tiles

# Schraudolph constants: code = round(s * SCH_A + SCH_B), bitcast bf16.
# SCH_A = 128*log2(e); SCH_B tuned so the multiplicative sawtooth error is
# zero-mean over uniform fractionals.
SCH_A = 184.66496736282892
SCH_B = 16248.67

_cache = {}
DBG_ALL_ACT = False
DBG_DUMP = False


def _build_program(sc_bufs=3, ypk_bufs=1, misc_bufs=1):
    import concourse.bass as bass
    import concourse.mybir as mybir
    import concourse.tile as tile
    from concourse import bacc

    f32 = mybir.dt.float32
    bf16 = mybir.dt.bfloat16
    i16 = mybir.dt.int16
    Exp = mybir.ActivationFunctionType.Exp
    mult = mybir.AluOpType.mult
    add = mybir.AluOpType.add

    nc = bacc.Bacc(
        "TRN2",
        target_bir_lowering=False,
        debug=False,
        enable_asserts=False,
        num_devices=N_CORES,
    )

    d_xT = nc.dram_tensor("xt_in", [D, S], bf16, kind="ExternalInput").ap()
    d_xTq = nc.dram_tensor("xtq_in", [D, 2048], bf16, kind="ExternalInput").ap()
    d_wq = nc.dram_tensor("wq_in", [D, GD], bf16, kind="ExternalInput").ap()
    d_wk = nc.dram_tensor("wk_in", [D, GD], bf16, kind="ExternalInput").ap()
    d_wv = nc.dram_tensor("wv_in", [D, GD], bf16, kind="ExternalInput").ap()
    d_bq = nc.dram_tensor("bq_in", [GD, 1], f32, kind="ExternalInput").ap()
    d_bk = nc.dram_tensor("bk_in", [GD, 1], f32, kind="ExternalInput").ap()
    d_bvb = nc.dram_tensor("bvb_in", [128, GD], f32, kind="ExternalInput").ap()
    d_wo = nc.dram_tensor("wo_in", [GD, D], bf16, kind="ExternalInput").ap()
    d_mm = nc.dram_tensor("mm_in", [2, 128, 128], bf16, kind="ExternalInput").ap()
    d_id = nc.dram_tensor("id_in", [128, 128], bf16, kind="ExternalInput").ap()
    d_out = nc.dram_tensor("outt_out", [D, 2048], f32, kind="ExternalOutput").ap()
    d_dbg = {}
    if DBG_DUMP:
        d_dbg["kt0"] = nc.dram_tensor("kt0_out", [128, S], bf16, kind="ExternalOutput").ap()
        d_dbg["kt1"] = nc.dram_tensor("kt1_out", [64, S], bf16, kind="ExternalOutput").ap()
        d_dbg["qt0"] = nc.dram_tensor("qt0_out", [128, 2048], bf16, kind="ExternalOutput").ap()
        d_dbg["qt1"] = nc.dram_tensor("qt1_out", [64, 2048], bf16, kind="ExternalOutput").ap()
        d_dbg["vb"] = nc.dram_tensor("vb_out", [128, 3, NK, 65], bf16, kind="ExternalOutput").ap()
        d_dbg["yt0"] = nc.dram_tensor("yt0_out", [128, 2048], bf16, kind="ExternalOutput").ap()
        d_dbg["yt1"] = nc.dram_tensor("yt1_out", [64, 2048], bf16, kind="ExternalOutput").ap()

    # scores kt groups: 2 key-tiles per psum buffer, 2-deep rotation
    groups = [(k0, 2) for k0 in range(0, NK, 2)]

    with tile.TileContext(nc) as tc:
        xTr = d_xT.rearrange("(k p) n -> p k n", p=128)     # [128, 6, 4096]
        xTqr = d_xTq.rearrange("(k p) n -> p k n", p=128)   # [128, 6, 2048]
        wqr = d_wq.rearrange("(k p) m -> p k m", p=128)
        wkr = d_wk.rearrange("(k p) m -> p k m", p=128)
        wvr = d_wv.rearrange("(k p) m -> p k m", p=128)
        mmr = d_mm.rearrange("m p n -> p m n")
        with tc.tile_pool(name="const", bufs=1) as cp:
            wq_sb = cp.tile([128, 6, GD], bf16, tag="wq")
            wk_sb = cp.tile([128, 6, GD], bf16, tag="wk")
            wv_sb = cp.tile([128, 6, GD], bf16, tag="wv")
            wo0 = cp.tile([128, D], bf16, tag="wo0")
            wo1 = cp.tile([64, D], bf16, tag="wo1")
            bq0 = cp.tile([128, 1], f32, tag="bq0")
            bq1 = cp.tile([64, 1], f32, tag="bq1")
            bk0 = cp.tile([128, 1], f32, tag="bk0")
            bk1 = cp.tile([64, 1], f32, tag="bk1")
            bvb = cp.tile([128, GD], f32, tag="bvb")
            mm_sb = cp.tile([128, 2, 128], bf16, tag="mm")
            id_sb = cp.tile([128, 128], bf16, tag="ident")
            ones = cp.tile([128, 1], bf16, tag="ones")
            zc = cp.tile([128, 65], bf16, tag="zc")
            KT0 = cp.tile([128, S], bf16, tag="KT0")
            KT1 = cp.tile([64, S], bf16, tag="KT1")
            QT0 = cp.tile([128, 2048], bf16, tag="QT0")
            QT1 = cp.tile([64, 2048], bf16, tag="QT1")
            Vb = cp.tile([128, 3, NK, 65], bf16, tag="Vb")
            yT0 = cp.tile([128, 2048], bf16, tag="yT0")
            yT1 = cp.tile([64, 2048], bf16, tag="yT1")
            pk0 = cp.tile([128, 16, 128], bf16, tag="pk0")  # packed ysb h0|h1

            nc.sync.dma_start(out=wq_sb[:], in_=wqr[:])
            nc.sync.dma_start(out=wk_sb[:], in_=wkr[:])
            nc.sync.dma_start(out=wv_sb[:], in_=wvr[:])
            nc.sync.dma_start(out=wo0[:], in_=d_wo[0:128, :])
            nc.sync.dma_start(out=wo1[:], in_=d_wo[128:GD, :])
            nc.sync.dma_start(out=bq0[:], in_=d_bq[0:128, :])
            nc.sync.dma_start(out=bq1[:], in_=d_bq[128:GD, :])
            nc.sync.dma_start(out=bk0[:], in_=d_bk[0:128, :])
            nc.sync.dma_start(out=bk1[:], in_=d_bk[128:GD, :])
            nc.sync.dma_start(out=bvb[:], in_=d_bvb[:, :])
            nc.sync.dma_start(out=mm_sb[:], in_=mmr[:])
            nc.sync.dma_start(out=id_sb[:], in_=d_id[:, :])
            nc.vector.memset(ones[:], 1.0)
            nc.vector.memset(zc[:], 0.0)
            nc.vector.memset(Vb[:, :, :, 64:65], 1.0)

            with (
                tc.tile_pool(name="xsl", bufs=4) as xp,
                tc.tile_pool(name="psA", bufs=sc_bufs, space="PSUM") as pa,
                tc.tile_pool(name="psK", bufs=ypk_bufs, space="PSUM") as pyk,
                tc.tile_pool(name="psM", bufs=misc_bufs, space="PSUM") as pm,
                tc.tile_pool(name="epool", bufs=2) as ep,
                tc.tile_pool(name="small", bufs=3) as sp,
                tc.tile_pool(name="ocp", bufs=4) as op_,
            ):
                def qproj(qb):
                    xq = xp.tile([128, 6, 512], bf16, tag="xq", name=f"xq{qb}")
                    nc.sync.dma_start(
                        out=xq[:], in_=xTqr[:, :, qb * 512:(qb + 1) * 512]
                    )
                    for i, (msz, off, QT_t, bq_t) in enumerate(
                        ((128, 0, QT0, bq0), (64, 128, QT1, bq1))
                    ):
                        ps = pm.tile([msz, 512], f32, tag="pm", name=f"qps{qb}_{i}")
                        for k in range(6):
                            nc.tensor.matmul(
                                ps[:],
                                wq_sb[:, k, off:off + msz],
                                xq[:, k, :],
                                start=(k == 0),
                                stop=(k == 5),
                            )
                        nc.vector.tensor_scalar_add(
                            QT_t[:, qb * 512:(qb + 1) * 512], ps[:], bq_t[:]
                        )

                def kslab(nb):
                    xs = xp.tile([128, 6, 512], bf16, tag="xq", name=f"xs{nb}")
                    nc.sync.dma_start(
                        out=xs[:], in_=xTr[:, :, nb * 512:(nb + 1) * 512]
                    )
                    for i, (msz, off, KT_t, bk_t) in enumerate(
                        ((128, 0, KT0, bk0), (64, 128, KT1, bk1))
                    ):
                        ps = pm.tile([msz, 512], f32, tag="pm", name=f"kps{nb}_{i}")
                        for k in range(6):
                            nc.tensor.matmul(
                                ps[:],
                                wk_sb[:, k, off:off + msz],
                                xs[:, k, :],
                                start=(k == 0),
                                stop=(k == 5),
                            )
                        nc.vector.tensor_scalar_add(
                            KT_t[:, nb * 512:(nb + 1) * 512], ps[:], bk_t[:]
                        )

                def vproj(nb):
                    xv = xp.tile([128, 6, 512], bf16, tag="xv", name=f"xv{nb}", bufs=3)
                    nc.sync.dma_start(
                        out=xv[:], in_=xTr[:, :, nb * 512:(nb + 1) * 512]
                    )
                    for ms in range(4):
                        kt = nb * 4 + ms
                        ps = pm.tile([128, GD], f32, tag="pm", name=f"vps{nb}_{ms}")
                        for k in range(6):
                            nc.tensor.matmul(
                                ps[:],
                                xv[:, k, ms * 128:(ms + 1) * 128],
                                wv_sb[:, k, :],
                                start=(k == 0),
                                stop=(k == 5),
                            )
                        nc.vector.tensor_tensor(
                            Vb[:, :, kt, 0:64],
                            ps[:].rearrange("p (h d) -> p h d", h=3),
                            bvb[:].rearrange("p (h d) -> p h d", h=3),
                            add,
                        )

                # ---- phase A head: Q(0) and K slabs 0-3 emitted directly ----
                qproj(0)
                for nb in range(4):
                    kslab(nb)

                # phase A tail, interleaved into the first units' score loops
                phase_a_tail = (
                    [lambda nb=nb: kslab(nb) for nb in range(4, 8)]
                    + [lambda qb=qb: qproj(qb) for qb in range(1, 4)]
                    + [lambda nb=nb: vproj(nb) for nb in range(8)]
                )

                def scores_emitters(qb, h):
                    """16 closures, one per 2-kt score group (matmuls + exp)."""
                    bnd = 8 * qb + 8
                    if h < 2:
                        KTh = KT0[64 * h:64 * (h + 1), :]
                        QTh = QT0[64 * h:64 * (h + 1), :]
                    else:
                        KTh = KT1[0:64, :]
                        QTh = QT1[0:64, :]
                    E = ep.tile([128, NK, 512], bf16, tag="E", name=f"E{qb}_{h}")
                    Ei = E[:].bitcast(i16)

                    def emit_group(k0, gsz):
                        ps = pa.tile(
                            [128, 2 * 512], f32, tag="psA", name=f"sc{qb}_{h}_{k0}"
                        )
                        for i in range(gsz):
                            kt = k0 + i
                            nc.tensor.matmul(
                                ps[:, i * 512:(i + 1) * 512],
                                KTh[:, kt * 128:(kt + 1) * 128],
                                QTh[:, qb * 512:(qb + 1) * 512],
                                start=True,
                                stop=True,
                            )
                        lo = min(max(bnd - k0, 0), gsz)
                        if DBG_ALL_ACT:
                            lo = gsz
                        if lo > 0:
                            nc.scalar.activation(
                                E[:, k0:k0 + lo, :], ps[:, 0:lo * 512], Exp
                            )
                        if lo < gsz:
                            nc.vector.tensor_scalar(
                                Ei[:, k0 + lo:k0 + gsz, :],
                                ps[:, lo * 512:gsz * 512],
                                SCH_A, SCH_B, mult, add,
                            )

                    # interleave ACT-destined (k0 < bnd) and DVE-destined
                    # groups so both exp engines run concurrently
                    a_list = [g for g in groups if g[0] < bnd]
                    u_list = [g for g in groups if g[0] >= bnd]
                    weave = []
                    na, nu = len(a_list), len(u_list)
                    ia = iu = 0
                    for i in range(na + nu):
                        # pace the shorter list evenly through the longer one
                        if ia * max(nu, 1) <= iu * max(na, 1) and ia < na or iu >= nu:
                            weave.append(a_list[ia]); ia += 1
                        else:
                            weave.append(u_list[iu]); iu += 1
                    return E, [
                        (lambda k0=k0, gsz=gsz: emit_group(k0, gsz))
                        for k0, gsz in weave
                    ]

                def post_emitters(qb, h, E):
                    """Consumer chunks for a finished unit: em + 4 chains.
                    All four 65-col chains plus the transpose region pack
                    into one psum tile (1 bank) to relieve bank pressure."""
                    chunks = []
                    em_all = {}
                    ypk = pyk.tile([128, 324], f32, tag="ypk", name=f"ypk{qb}_{h}")

                    def emit_em(tl):
                        t = qb * 4 + tl
                        tsl = slice(tl * 128, (tl + 1) * 128)
                        for j in range(2):
                            em = sp.tile([128, 128], bf16, tag="em",
                                         name=f"em{t}_{h}_{j}", bufs=10)
                            nc.vector.tensor_tensor(
                                em[:], E[:, 2 * t + j, tsl], mm_sb[:, j, :], mult
                            )
                            em_all[(tl, j)] = em

                    def emit_chain(tl):
                        t = qb * 4 + tl
                        tsl = slice(tl * 128, (tl + 1) * 128)
                        o = tl * 65
                        ypn = ypk[:, o:o + 64]
                        ypd = ypk[:, o + 64:o + 65]
                        ypf = ypk[:, o:o + 65]
                        if t == 0:
                            # zero the whole 65-col region in one start group
                            nc.tensor.matmul(ypf, E[:, 0, tsl], zc[:],
                                             start=True, stop=False)
                        for kt in range(2 * t):
                            nc.tensor.matmul(
                                ypf, E[:, kt, tsl], Vb[:, h, kt, :],
                                start=(kt == 0), stop=False,
                            )
                        for j in range(2):
                            kt = 2 * t + j
                            nc.tensor.matmul(
                                ypn, em_all[(tl, j)][:],
                                Vb[:, h, kt, 0:64],
                                start=False, stop=False,
                            )
                            nc.tensor.matmul(
                                ypd, E[:, kt, tsl], ones[:],
                                start=False,
                                stop=(t == 15 and j == 1),
                            )
                        for kt in range(2 * t + 2, NK):
                            nc.tensor.matmul(
                                ypd, E[:, kt, tsl], ones[:],
                                start=False, stop=(kt == NK - 1),
                            )
                        rc = sp.tile([128, 1], f32, tag="rc",
                                     name=f"rc{t}_{h}", bufs=6)
                        nc.vector.reciprocal(rc[:], ypd)
                        if h < 2:
                            dst = pk0[:, t, 64 * h:64 * (h + 1)]
                        else:
                            dst = sp.tile([128, 64], bf16, tag="ysb2",
                                          name=f"ysb2{t}", bufs=3)
                        nc.vector.tensor_scalar_mul(dst, ypn, rc[:])
                        if h == 1:
                            tp = ypk[:, 260:324].bitcast(bf16)
                            nc.tensor.transpose(tp, pk0[:, t, :], id_sb[:])
                            nc.vector.tensor_copy(
                                yT0[:, t * 128:(t + 1) * 128], tp
                            )
                        elif h == 2:
                            tp = ypk[0:64, 260:324].bitcast(bf16)
                            nc.tensor.transpose(tp, dst, id_sb[:])
                            nc.vector.tensor_copy(
                                yT1[:, t * 128:(t + 1) * 128], tp
                            )

                    for tl in range(4):
                        chunks.append(lambda tl=tl: emit_em(tl))
                        chunks.append(lambda tl=tl: emit_chain(tl))
                    return chunks

                def wout_emitters(qb):
                    def emit_wout(mt):
                        csl = slice(qb * 512, (qb + 1) * 512)
                        ps = pm.tile([128, 512], f32, tag="pm", name=f"ops{qb}_{mt}")
                        nc.tensor.matmul(
                            ps[:], wo0[:, mt * 128:(mt + 1) * 128], yT0[:, csl],
                            start=True, stop=False,
                        )
                        nc.tensor.matmul(
                            ps[:], wo1[:, mt * 128:(mt + 1) * 128], yT1[:, csl],
                            start=False, stop=True,
                        )
                        oc = op_.tile([128, 512], f32, tag="ocp", name=f"oc{qb}_{mt}")
                        nc.scalar.copy(oc[:], ps[:])
                        nc.sync.dma_start(
                            out=d_out[mt * 128:(mt + 1) * 128, csl], in_=oc[:]
                        )

                    return [lambda mt=mt: emit_wout(mt) for mt in range(6)]

                # ---- software-pipelined emission over the 12 units ----
                units = [(qb, h) for qb in range(4) for h in range(3)]
                pending = list(phase_a_tail)
                for qb, h in units:
                    E, sc = scores_emitters(qb, h)
                    done = 0
                    n = len(pending)
                    for g, cb in enumerate(sc):
                        cb()
                        want = (n * (g + 1) + 15) // 16
                        while done < min(want, n):
                            pending[done]()
                            done += 1
                    while done < n:
                        pending[done]()
                        done += 1
                    pending = post_emitters(qb, h, E)
                    if h == 2:
                        pending.extend(wout_emitters(qb))
                for cb in pending:
                    cb()
                if DBG_DUMP:
                    nc.sync.dma_start(out=d_dbg["kt0"][:, :], in_=KT0[:])
                    nc.sync.dma_start(out=d_dbg["kt1"][:, :], in_=KT1[:])
                    nc.sync.dma_start(out=d_dbg["qt0"][:, :], in_=QT0[:])
                    nc.sync.dma_start(out=d_dbg["qt1"][:, :], in_=QT1[:])
                    nc.sync.dma_start(out=d_dbg["vb"][:, :, :, :], in_=Vb[:])
                    nc.sync.dma_start(out=d_dbg["yt0"][:, :], in_=yT0[:])
                    nc.sync.dma_start(out=d_dbg["yt1"][:, :], in_=yT1[:])

    nc.compile()
    return nc


def _get_program(sc_bufs=3, ypk_bufs=1, misc_bufs=1):
    key = ("nc", sc_bufs, ypk_bufs, misc_bufs)
    if key not in _cache:
        _cache[key] = _build_program(sc_bufs, ypk_bufs, misc_bufs)
    return _cache[key]


def _qcols(s):
    idx = []
    for t in range(LT):
        g = 2 * t + s
        idx.extend(range(g * 128, g * 128 + 128))
    return np.array(idx)


def _masks(s):
    # em = E[k_part, q_free] * M_j for band tile kt = 2t + j, q-tile g = 2t+s
    tri = (np.arange(128)[:, None] <= np.arange(128)[None, :]).astype(np.float32)
    M = np.zeros((2, 128, 128), np.float32)
    if s == 0:
        M[0] = tri          # kt == g
        M[1] = 0.0          # kt == g+1 (future)
    else:
        M[0] = 1.0          # kt == g-1 (past)
        M[1] = tri          # kt == g
    return M


def shard_inputs(x, W_qkv, b_qkv, W_out, b_out):
    """Build the 8 per-core input maps."""
    bf = ml_dtypes.bfloat16
    xT = np.ascontiguousarray(x[0].T.astype(np.float32)).astype(bf)  # [D, S]
    ident = np.eye(128, dtype=np.float32).astype(bf)
    in_maps = []
    per_s = {}
    for s in (0, 1):
        cols = _qcols(s)
        per_s[s] = (
            np.ascontiguousarray(xT[:, cols]),
            _masks(s).astype(bf),
        )
    for c in range(N_CORES):
        hg, s = c // 2, c % 2
        hsl = slice(GD * hg, GD * (hg + 1))
        xTq, M = per_s[s]
        wq = np.ascontiguousarray((W_qkv[0:768][hsl] / 8.0).T.astype(np.float32)).astype(bf)
        wk = np.ascontiguousarray(W_qkv[768:1536][hsl].T.astype(np.float32)).astype(bf)
        wv = np.ascontiguousarray(W_qkv[1536:2304][hsl].T.astype(np.float32)).astype(bf)
        bq = (b_qkv[0:768][hsl] / 8.0).astype(np.float32).reshape(GD, 1)
        bk = b_qkv[768:1536][hsl].astype(np.float32).reshape(GD, 1)
        bv = b_qkv[1536:2304][hsl].astype(np.float32)
        bvb = np.ascontiguousarray(np.broadcast_to(bv[None, :], (128, GD)))
        wo = np.ascontiguousarray(W_out[:, hsl].T.astype(np.float32)).astype(bf)
        in_maps.append(
            {
                "xt_in": xT,
                "xtq_in": xTq,
                "wq_in": wq,
                "wk_in": wk,
                "wv_in": wv,
                "bq_in": bq,
                "bk_in": bk,
                "bvb_in": bvb,
                "wo_in": wo,
                "mm_in": M,
                "id_in": ident,
            }
        )
    return in_maps


def gather_output(results, b_out):
    out = np.zeros((S, D), np.float32)
    for s in (0, 1):
        acc = np.zeros((2048, D), np.float32)
        for hg in range(4):
            c = hg * 2 + s
            acc += results[c]["outt_out"].T
        out[_qcols(s)] = acc + b_out[None, :].astype(np.float32)
    return out.reshape(1, S, D)


def kernel(x, W_qkv, b_qkv, W_out, b_out):
    from concourse.bass_utils import run_bass_kernel_spmd

    x = np.asarray(x)
    W_qkv = np.asarray(W_qkv)
    b_qkv = np.asarray(b_qkv)
    W_out = np.asarray(W_out)
    b_out = np.asarray(b_out)
    nc = _get_program()
    in_maps = shard_inputs(x, W_qkv, b_qkv, W_out, b_out)
    res = run_bass_kernel_spmd(nc, in_maps, list(range(N_CORES)))
    return gather_output(res.results, b_out)


# revision 29
# speedup vs baseline: 1.1408x; 1.0961x over previous
"""Trainium2 Bass kernel for nn_MinimalAttention (B=1, S=4096, d_model=768,
H=12, Dh=64, post-softmax causal mask).

Sharding (8 cores): 4 head-groups (3 heads) x 2 sequence shards.
Seq shard s owns the 2048 q rows with tile-index g === s (mod 2): local
q-subtile t (16 per core, 128 rows) maps to global q-tile g = 2t + s, so
the causal structure (2t full key-tiles, 2 masked band tiles, rest
strictly-upper) is identical on every core; all per-core variation is
input data (weight slices, gathered xTq columns, 2 mask tiles).

Engine split per core:
  PE     : K/Q/V projections, scores^T = K_h^T x Q_h^T per 128-key tile
           (bf16, PSUM f32), numerator y[q,0:65] += E^T @ [V|1] with the
           fused ones column accumulating the denominator, ~free 1-column
           ones-matmuls adding the upper-region denominator, packed
           transposes, output projection.
  ACT    : exact exp for the causal lower + diagonal band region.
  DVE    : Schraudolph bit-trick exp (f32 -> int16 code, bitcast bf16,
           ~+-3% sawtooth, zero-mean tuned) for the strictly-upper region
           which only feeds the softmax denominator (error averages out
           over ~2k keys), diagonal masking (em), normalize, psum->SBUF
           copies with fused bias.

Emission is software-pipelined: the post-exp work of unit i-1 (numerator
chains, normalize, transposes) and the phase-A tail (K slabs 4-7, Q 1-3,
V projection) are interleaved between the score groups of unit i so the
in-order PE queue never parks on a not-yet-exp'd psum.

Host sums the 4 head-group partials per shard, adds b_out, and scatters
the mod-2 interleaved rows back.
"""
import sys

sys.path.insert(0, "/opt/trn_rl_repo")

import numpy as np
import ml_dtypes

S, D, H, DH = 4096, 768, 12, 64
N_CORES = 8
GD = 192          # head-group dims (3 heads)
LT = 16           # local 128-row subtiles per core (2048 q rows)
NK = 32           # key tiles

# Schraudolph constants: code = round(s * SCH_A + SCH_B), bitcast bf16.
# SCH_A = 128*log2(e); SCH_B tuned so the multiplicative sawtooth error is
# zero-mean over uniform fractionals.
SCH_A = 184.66496736282892
SCH_B = 16248.67

_cache = {}
DBG_ALL_ACT = False
DBG_DUMP = False


def _build_program(sc_bufs=3, ypk_bufs=1, misc_bufs=1):
    import concourse.bass as bass
    import concourse.mybir as mybir
    import concourse.tile as tile
    from concourse import bacc

    f32 = mybir.dt.float32
    bf16 = mybir.dt.bfloat16
    i16 = mybir.dt.int16
    Exp = mybir.ActivationFunctionType.Exp
    mult = mybir.AluOpType.mult
    add = mybir.AluOpType.add

    nc = bacc.Bacc(
        "TRN2",
        target_bir_lowering=False,
        debug=False,
        enable_asserts=False,
        num_devices=N_CORES,
    )

    d_xT = nc.dram_tensor("xt_in", [D, S], bf16, kind="ExternalInput").ap()
    d_xTq = nc.dram_tensor("xtq_in", [D, 2048], bf16, kind="ExternalInput").ap()
    d_wq = nc.dram_tensor("wq_in", [D, GD], bf16, kind="ExternalInput").ap()
    d_wk = nc.dram_tensor("wk_in", [D, GD], bf16, kind="ExternalInput").ap()
    d_wv = nc.dram_tensor("wv_in", [D, GD], bf16, kind="ExternalInput").ap()
    d_bq = nc.dram_tensor("bq_in", [GD, 1], f32, kind="ExternalInput").ap()
    d_bk = nc.dram_tensor("bk_in", [GD, 1], f32, kind="ExternalInput").ap()
    d_bvb = nc.dram_tensor("bvb_in", [128, GD], f32, kind="ExternalInput").ap()
    d_wo = nc.dram_tensor("wo_in", [GD, D], bf16, kind="ExternalInput").ap()
    d_mm = nc.dram_tensor("mm_in", [2, 128, 128], bf16, kind="ExternalInput").ap()
    d_id = nc.dram_tensor("id_in", [128, 128], bf16, kind="ExternalInput").ap()
    d_out = nc.dram_tensor("outt_out", [D, 2048], f32, kind="ExternalOutput").ap()
    d_dbg = {}
    if DBG_DUMP:
        d_dbg["kt0"] = nc.dram_tensor("kt0_out", [128, S], bf16, kind="ExternalOutput").ap()
        d_dbg["kt1"] = nc.dram_tensor("kt1_out", [64, S], bf16, kind="ExternalOutput").ap()
        d_dbg["qt0"] = nc.dram_tensor("qt0_out", [128, 2048], bf16, kind="ExternalOutput").ap()
        d_dbg["qt1"] = nc.dram_tensor("qt1_out", [64, 2048], bf16, kind="ExternalOutput").ap()
        d_dbg["vb"] = nc.dram_tensor("vb_out", [128, 3, NK, 65], bf16, kind="ExternalOutput").ap()
        d_dbg["yt0"] = nc.dram_tensor("yt0_out", [128, 2048], bf16, kind="ExternalOutput").ap()
        d_dbg["yt1"] = nc.dram_tensor("yt1_out", [64, 2048], bf16, kind="ExternalOutput").ap()

    # scores kt groups: 2 key-tiles per psum buffer, 2-deep rotation
    groups = [(k0, 2) for k0 in range(0, NK, 2)]

    with tile.TileContext(nc) as tc:
        xTr = d_xT.rearrange("(k p) n -> p k n", p=128)     # [128, 6, 4096]
        xTqr = d_xTq.rearrange("(k p) n -> p k n", p=128)   # [128, 6, 2048]
        wqr = d_wq.rearrange("(k p) m -> p k m", p=128)
        wkr = d_wk.rearrange("(k p) m -> p k m", p=128)
        wvr = d_wv.rearrange("(k p) m -> p k m", p=128)
        mmr = d_mm.rearrange("m p n -> p m n")
        with tc.tile_pool(name="const", bufs=1) as cp:
            wq_sb = cp.tile([128, 6, GD], bf16, tag="wq")
            wk_sb = cp.tile([128, 6, GD], bf16, tag="wk")
            wv_sb = cp.tile([128, 6, GD], bf16, tag="wv")
            wo0 = cp.tile([128, D], bf16, tag="wo0")
            wo1 = cp.tile([64, D], bf16, tag="wo1")
            bq0 = cp.tile([128, 1], f32, tag="bq0")
            bq1 = cp.tile([64, 1], f32, tag="bq1")
            bk0 = cp.tile([128, 1], f32, tag="bk0")
            bk1 = cp.tile([64, 1], f32, tag="bk1")
            bvb = cp.tile([128, GD], f32, tag="bvb")
            mm_sb = cp.tile([128, 2, 128], bf16, tag="mm")
            id_sb = cp.tile([128, 128], bf16, tag="ident")
            ones = cp.tile([128, 1], bf16, tag="ones")
            zc = cp.tile([128, 65], bf16, tag="zc")
            KT0 = cp.tile([128, S], bf16, tag="KT0")
            KT1 = cp.tile([64, S], bf16, tag="KT1")
            QT0 = cp.tile([128, 2048], bf16, tag="QT0")
            QT1 = cp.tile([64, 2048], bf16, tag="QT1")
            Vb = cp.tile([128, 3, NK, 65], bf16, tag="Vb")
            yT0 = cp.tile([128, 2048], bf16, tag="yT0")
            yT1 = cp.tile([64, 2048], bf16, tag="yT1")
            pk0 = cp.tile([128, 16, 128], bf16, tag="pk0")  # packed ysb h0|h1

            nc.sync.dma_start(out=wq_sb[:], in_=wqr[:])
            nc.sync.dma_start(out=bq0[:], in_=d_bq[0:128, :])
            nc.sync.dma_start(out=bq1[:], in_=d_bq[128:GD, :])
            nc.sync.dma_start(out=wk_sb[:], in_=wkr[:])
            nc.sync.dma_start(out=bk0[:], in_=d_bk[0:128, :])
            nc.sync.dma_start(out=bk1[:], in_=d_bk[128:GD, :])
            nc.vector.memset(ones[:], 1.0)
            nc.vector.memset(zc[:], 0.0)
            nc.vector.memset(Vb[:, :, :, 64:65], 1.0)

            def late_const_dmas():
                nc.sync.dma_start(out=wv_sb[:], in_=wvr[:])
                nc.sync.dma_start(out=mm_sb[:], in_=mmr[:])
                nc.sync.dma_start(out=id_sb[:], in_=d_id[:, :])
                nc.sync.dma_start(out=bvb[:], in_=d_bvb[:, :])
                nc.sync.dma_start(out=wo0[:], in_=d_wo[0:128, :])
                nc.sync.dma_start(out=wo1[:], in_=d_wo[128:GD, :])

            with (
                tc.tile_pool(name="xsl", bufs=5) as xp,
                tc.tile_pool(name="psA", bufs=sc_bufs, space="PSUM") as pa,
                tc.tile_pool(name="psK", bufs=ypk_bufs, space="PSUM") as pyk,
                tc.tile_pool(name="psM", bufs=misc_bufs, space="PSUM") as pm,
                tc.tile_pool(name="epool", bufs=3) as ep,
                tc.tile_pool(name="small", bufs=3) as sp,
                tc.tile_pool(name="ocp", bufs=3) as op_,
            ):
                def qproj(qb):
                    xq = xp.tile([128, 6, 512], bf16, tag="xq", name=f"xq{qb}")
                    nc.sync.dma_start(
                        out=xq[:], in_=xTqr[:, :, qb * 512:(qb + 1) * 512]
                    )
                    for i, (msz, off, QT_t, bq_t) in enumerate(
                        ((128, 0, QT0, bq0), (64, 128, QT1, bq1))
                    ):
                        ps = pm.tile([msz, 512], f32, tag="pm", name=f"qps{qb}_{i}")
                        for k in range(6):
                            nc.tensor.matmul(
                                ps[:],
                                wq_sb[:, k, off:off + msz],
                                xq[:, k, :],
                                start=(k == 0),
                                stop=(k == 5),
                            )
                        nc.vector.tensor_scalar_add(
                            QT_t[:, qb * 512:(qb + 1) * 512], ps[:], bq_t[:]
                        )

                def kslab(nb):
                    xs = xp.tile([128, 6, 512], bf16, tag="xq", name=f"xs{nb}")
                    nc.sync.dma_start(
                        out=xs[:], in_=xTr[:, :, nb * 512:(nb + 1) * 512]
                    )
                    for i, (msz, off, KT_t, bk_t) in enumerate(
                        ((128, 0, KT0, bk0), (64, 128, KT1, bk1))
                    ):
                        ps = pm.tile([msz, 512], f32, tag="pm", name=f"kps{nb}_{i}")
                        for k in range(6):
                            nc.tensor.matmul(
                                ps[:],
                                wk_sb[:, k, off:off + msz],
                                xs[:, k, :],
                                start=(k == 0),
                                stop=(k == 5),
                            )
                        nc.vector.tensor_scalar_add(
                            KT_t[:, nb * 512:(nb + 1) * 512], ps[:], bk_t[:]
                        )

                def vproj(nb):
                    xv = xp.tile([128, 6, 512], bf16, tag="xv", name=f"xv{nb}", bufs=2)
                    nc.sync.dma_start(
                        out=xv[:], in_=xTr[:, :, nb * 512:(nb + 1) * 512]
                    )
                    for ms in range(4):
                        kt = nb * 4 + ms
                        ps = pm.tile([128, GD], f32, tag="pm", name=f"vps{nb}_{ms}")
                        for k in range(6):
                            nc.tensor.matmul(
                                ps[:],
                                xv[:, k, ms * 128:(ms + 1) * 128],
                                wv_sb[:, k, :],
                                start=(k == 0),
                                stop=(k == 5),
                            )
                        nc.vector.tensor_tensor(
                            Vb[:, :, kt, 0:64],
                            ps[:].rearrange("p (h d) -> p h d", h=3),
                            bvb[:].rearrange("p (h d) -> p h d", h=3),
                            add,
                        )

                # ---- phase A head: Q(0) and K slabs 0-3 emitted directly ----
                qproj(0)
                for nb in range(4):
                    kslab(nb)

                # phase A tail: K slabs 4-7 woven into the first unit (its
                # upper groups need them progressively); Q/V spread over the
                # following positions, early enough for each consumer.
                late_const_dmas()
                phase_a_assign = {
                    0: [lambda nb=nb: kslab(nb) for nb in range(4, 8)],
                    1: [lambda: qproj(3), lambda: vproj(0), lambda: vproj(1)],
                    2: [lambda: vproj(2), lambda: vproj(3), lambda: qproj(1)],
                    3: [lambda: vproj(4), lambda: vproj(5)],
                    4: [lambda: vproj(6), lambda: vproj(7), lambda: qproj(2)],
                }

                def scores_emitters(qb, h, first=False):
                    """16 closures, one per 2-kt score group (matmuls + exp)."""
                    bnd = 8 * qb + 8
                    if h < 2:
                        KTh = KT0[64 * h:64 * (h + 1), :]
                        QTh = QT0[64 * h:64 * (h + 1), :]
                    else:
                        KTh = KT1[0:64, :]
                        QTh = QT1[0:64, :]
                    E = ep.tile([128, NK, 512], bf16, tag="E", name=f"E{qb}_{h}")
                    Ei = E[:].bitcast(i16)

                    def emit_group(k0, gsz):
                        ps = pa.tile(
                            [128, 2 * 512], f32, tag="psA", name=f"sc{qb}_{h}_{k0}"
                        )
                        for i in range(gsz):
                            kt = k0 + i
                            nc.tensor.matmul(
                                ps[:, i * 512:(i + 1) * 512],
                                KTh[:, kt * 128:(kt + 1) * 128],
                                QTh[:, qb * 512:(qb + 1) * 512],
                                start=True,
                                stop=True,
                            )
                        lo = min(max(bnd - k0, 0), gsz)
                        if DBG_ALL_ACT:
                            lo = gsz
                        if lo > 0:
                            nc.scalar.activation(
                                E[:, k0:k0 + lo, :], ps[:, 0:lo * 512], Exp
                            )
                        if lo < gsz:
                            nc.vector.tensor_scalar(
                                Ei[:, k0 + lo:k0 + gsz, :],
                                ps[:, lo * 512:gsz * 512],
                                SCH_A, SCH_B, mult, add,
                            )

                    # interleave ACT-destined (k0 < bnd) and DVE-destined
                    # groups so both exp engines run concurrently
                    a_list = [g for g in groups if g[0] < bnd]
                    u_list = [g for g in groups if g[0] >= bnd]
                    weave = []
                    na, nu = len(a_list), len(u_list)
                    ia = iu = 0
                    for i in range(na + nu):
                        # pace the shorter list evenly through the longer one
                        if ia * max(nu, 1) <= iu * max(na, 1) and ia < na or iu >= nu:
                            weave.append(a_list[ia]); ia += 1
                        else:
                            weave.append(u_list[iu]); iu += 1
                    if first:
                        weave = groups  # ascending: K slabs arrive in order
                    return E, [
                        (lambda k0=k0, gsz=gsz: emit_group(k0, gsz))
                        for k0, gsz in weave
                    ]

                def post_emitters(qb, h, E):
                    """Consumer chunks for a finished unit: em + 4 chains.
                    All four 65-col chains plus the transpose region pack
                    into one psum tile (1 bank) to relieve bank pressure."""
                    chunks = []
                    em_all = {}
                    ypk = pyk.tile([128, 324], f32, tag="ypk", name=f"ypk{qb}_{h}")

                    def emit_em(tl):
                        t = qb * 4 + tl
                        tsl = slice(tl * 128, (tl + 1) * 128)
                        for j in range(2):
                            em = sp.tile([128, 128], bf16, tag="em",
                                         name=f"em{t}_{h}_{j}", bufs=10)
                            nc.gpsimd.tensor_tensor(
                                em[:], E[:, 2 * t + j, tsl], mm_sb[:, j, :], mult
                            )
                            em_all[(tl, j)] = em

                    def emit_chain(tl):
                        t = qb * 4 + tl
                        tsl = slice(tl * 128, (tl + 1) * 128)
                        o = tl * 65
                        ypn = ypk[:, o:o + 64]
                        ypd = ypk[:, o + 64:o + 65]
                        ypf = ypk[:, o:o + 65]
                        if t == 0:
                            # zero the whole 65-col region in one start group
                            nc.tensor.matmul(ypf, E[:, 0, tsl], zc[:],
                                             start=True, stop=False)
                        for kt in range(2 * t):
                            nc.tensor.matmul(
                                ypf, E[:, kt, tsl], Vb[:, h, kt, :],
                                start=(kt == 0), stop=False,
                            )
                        for j in range(2):
                            kt = 2 * t + j
                            nc.tensor.matmul(
                                ypn, em_all[(tl, j)][:],
                                Vb[:, h, kt, 0:64],
                                start=False, stop=False,
                            )
                            nc.tensor.matmul(
                                ypd, E[:, kt, tsl], ones[:],
                                start=False,
                                stop=(t == 15 and j == 1),
                            )
                        for kt in range(2 * t + 2, NK):
                            nc.tensor.matmul(
                                ypd, E[:, kt, tsl], ones[:],
                                start=False, stop=(kt == NK - 1),
                            )

                    rc4 = sp.tile([128, 4], f32, tag="rc",
                                  name=f"rc4_{qb}_{h}", bufs=3)

                    def emit_fin(tl):
                        t = qb * 4 + tl
                        o = tl * 65
                        ypn = ypk[:, o:o + 64]
                        if tl == 0:
                            nc.vector.reciprocal(
                                rc4[:], ypk[:, 64:65 + 3 * 65:65]
                            )
                        if h < 2:
                            dst = pk0[:, t, 64 * h:64 * (h + 1)]
                        else:
                            dst = sp.tile([128, 64], bf16, tag="ysb2",
                                          name=f"ysb2{t}", bufs=3)
                        nc.vector.tensor_scalar_mul(dst, ypn, rc4[:, tl:tl + 1])
                        if h == 1:
                            tp = ypk[:, 260:324].bitcast(bf16)
                            nc.tensor.transpose(tp, pk0[:, t, :], id_sb[:])
                            nc.vector.tensor_copy(
                                yT0[:, t * 128:(t + 1) * 128], tp
                            )
                        elif h == 2:
                            tp = ypk[0:64, 260:324].bitcast(bf16)
                            nc.tensor.transpose(tp, dst, id_sb[:])
                            nc.vector.tensor_copy(
                                yT1[:, t * 128:(t + 1) * 128], tp
                            )

                    for tl in range(4):
                        chunks.append(lambda tl=tl: emit_em(tl))
                        chunks.append(lambda tl=tl: emit_chain(tl))
                    fins = [lambda tl=tl: emit_fin(tl) for tl in range(4)]
                    return chunks, fins

                def wout_emitters(qb):
                    def emit_wout(mt):
                        csl = slice(qb * 512, (qb + 1) * 512)
                        ps = pm.tile([128, 512], f32, tag="pm", name=f"ops{qb}_{mt}")
                        nc.tensor.matmul(
                            ps[:], wo0[:, mt * 128:(mt + 1) * 128], yT0[:, csl],
                            start=True, stop=False,
                        )
                        nc.tensor.matmul(
                            ps[:], wo1[:, mt * 128:(mt + 1) * 128], yT1[:, csl],
                            start=False, stop=True,
                        )
                        oc = op_.tile([128, 512], f32, tag="ocp", name=f"oc{qb}_{mt}")
                        nc.scalar.copy(oc[:], ps[:])
                        nc.sync.dma_start(
                            out=d_out[mt * 128:(mt + 1) * 128, csl], in_=oc[:]
                        )

                    return [lambda mt=mt: emit_wout(mt) for mt in range(6)]

                # ---- software-pipelined emission over the 12 units ----
                # Unit order pairs ACT-heavy (qb=3, all-exact-exp) with
                # DVE-heavy (qb=0) units so both exp engines stay fed.
                # chains of a unit are emitted 1 position later (2 for qb=3,
                # whose V/Q arrive later); fins one position after chains.
                units = [(0, 0), (0, 1), (3, 0), (0, 2), (3, 1), (1, 0),
                         (3, 2), (1, 1), (2, 0), (1, 2), (2, 1), (2, 2)]
                npos = len(units)
                sched = {p: list(phase_a_assign.get(p, [])) for p in range(npos + 3)}
                for pos, (qb, h) in enumerate(units):
                    E, sc = scores_emitters(qb, h, first=(pos == 0))
                    chains, fins = post_emitters(qb, h, E)
                    lag = 2 if qb == 3 else 1
                    sched[min(pos + lag, npos + 1)].extend(chains)
                    fl = sched[min(pos + lag + 1, npos + 2)]
                    fl.extend(fins)
                    if h == 2:
                        fl.extend(wout_emitters(qb))
                    pending = sched[pos]
                    done = 0
                    n = len(pending)
                    for g, cb in enumerate(sc):
                        cb()
                        want = (n * (g + 1) + 15) // 16
                        while done < min(want, n):
                            pending[done]()
                            done += 1
                    while done < n:
                        pending[done]()
                        done += 1
                for p in range(npos, npos + 3):
                    for cb in sched[p]:
                        cb()
                if DBG_DUMP:
                    nc.sync.dma_start(out=d_dbg["kt0"][:, :], in_=KT0[:])
                    nc.sync.dma_start(out=d_dbg["kt1"][:, :], in_=KT1[:])
                    nc.sync.dma_start(out=d_dbg["qt0"][:, :], in_=QT0[:])
                    nc.sync.dma_start(out=d_dbg["qt1"][:, :], in_=QT1[:])
                    nc.sync.dma_start(out=d_dbg["vb"][:, :, :, :], in_=Vb[:])
                    nc.sync.dma_start(out=d_dbg["yt0"][:, :], in_=yT0[:])
                    nc.sync.dma_start(out=d_dbg["yt1"][:, :], in_=yT1[:])

    nc.compile()
    return nc


def _get_program(sc_bufs=3, ypk_bufs=1, misc_bufs=1):
    key = ("nc", sc_bufs, ypk_bufs, misc_bufs)
    if key not in _cache:
        _cache[key] = _build_program(sc_bufs, ypk_bufs, misc_bufs)
    return _cache[key]


def _qcols(s):
    idx = []
    for t in range(LT):
        g = 2 * t + s
        idx.extend(range(g * 128, g * 128 + 128))
    return np.array(idx)


def _masks(s):
    # em = E[k_part, q_free] * M_j for band tile kt = 2t + j, q-tile g = 2t+s
    tri = (np.arange(128)[:, None] <= np.arange(128)[None, :]).astype(np.float32)
    M = np.zeros((2, 128, 128), np.float32)
    if s == 0:
        M[0] = tri          # kt == g
        M[1] = 0.0          # kt == g+1 (future)
    else:
        M[0] = 1.0          # kt == g-1 (past)
        M[1] = tri          # kt == g
    return M


def shard_inputs(x, W_qkv, b_qkv, W_out, b_out):
    """Build the 8 per-core input maps."""
    bf = ml_dtypes.bfloat16
    xT = np.ascontiguousarray(x[0].T.astype(np.float32)).astype(bf)  # [D, S]
    ident = np.eye(128, dtype=np.float32).astype(bf)
    in_maps = []
    per_s = {}
    for s in (0, 1):
        cols = _qcols(s)
        per_s[s] = (
            np.ascontiguousarray(xT[:, cols]),
            _masks(s).astype(bf),
        )
    for c in range(N_CORES):
        hg, s = c // 2, c % 2
        hsl = slice(GD * hg, GD * (hg + 1))
        xTq, M = per_s[s]
        wq = np.ascontiguousarray((W_qkv[0:768][hsl] / 8.0).T.astype(np.float32)).astype(bf)
        wk = np.ascontiguousarray(W_qkv[768:1536][hsl].T.astype(np.float32)).astype(bf)
        wv = np.ascontiguousarray(W_qkv[1536:2304][hsl].T.astype(np.float32)).astype(bf)
        bq = (b_qkv[0:768][hsl] / 8.0).astype(np.float32).reshape(GD, 1)
        bk = b_qkv[768:1536][hsl].astype(np.float32).reshape(GD, 1)
        bv = b_qkv[1536:2304][hsl].astype(np.float32)
        bvb = np.ascontiguousarray(np.broadcast_to(bv[None, :], (128, GD)))
        wo = np.ascontiguousarray(W_out[:, hsl].T.astype(np.float32)).astype(bf)
        in_maps.append(
            {
                "xt_in": xT,
                "xtq_in": xTq,
                "wq_in": wq,
                "wk_in": wk,
                "wv_in": wv,
                "bq_in": bq,
                "bk_in": bk,
                "bvb_in": bvb,
                "wo_in": wo,
                "mm_in": M,
                "id_in": ident,
            }
        )
    return in_maps


def gather_output(results, b_out):
    out = np.zeros((S, D), np.float32)
    for s in (0, 1):
        acc = np.zeros((2048, D), np.float32)
        for hg in range(4):
            c = hg * 2 + s
            acc += results[c]["outt_out"].T
        out[_qcols(s)] = acc + b_out[None, :].astype(np.float32)
    return out.reshape(1, S, D)


def kernel(x, W_qkv, b_qkv, W_out, b_out):
    from concourse.bass_utils import run_bass_kernel_spmd

    x = np.asarray(x)
    W_qkv = np.asarray(W_qkv)
    b_qkv = np.asarray(b_qkv)
    W_out = np.asarray(W_out)
    b_out = np.asarray(b_out)
    nc = _get_program()
    in_maps = shard_inputs(x, W_qkv, b_qkv, W_out, b_out)
    res = run_bass_kernel_spmd(nc, in_maps, list(range(N_CORES)))
    return gather_output(res.results, b_out)


# revision 38
# speedup vs baseline: 1.1902x; 1.0433x over previous
"""Trainium2 Bass kernel for nn_MinimalAttention (B=1, S=4096, d_model=768,
H=12, Dh=64, post-softmax causal mask).

Sharding (8 cores): 4 head-groups (3 heads) x 2 sequence shards.
Seq shard s owns the 2048 q rows with tile-index g === s (mod 2): local
q-subtile t (16 per core, 128 rows) maps to global q-tile g = 2t + s, so
the causal structure (2t full key-tiles, 2 masked band tiles, rest
strictly-upper) is identical on every core; all per-core variation is
input data (weight slices, gathered xTq columns, 2 mask tiles).

Engine split per core:
  PE     : K/Q/V projections, scores^T = K_h^T x Q_h^T per 128-key tile
           (bf16, PSUM f32), numerator y[q,0:65] += E^T @ [V|1] with the
           fused ones column accumulating the denominator, ~free 1-column
           ones-matmuls adding the upper-region denominator, packed
           transposes, output projection.
  ACT    : exact exp for the causal lower + diagonal band region.
  DVE    : Schraudolph bit-trick exp (f32 -> int16 code, bitcast bf16,
           ~+-3% sawtooth, zero-mean tuned) for the strictly-upper region
           which only feeds the softmax denominator (error averages out
           over ~2k keys), diagonal masking (em), normalize, psum->SBUF
           copies with fused bias.

Emission is software-pipelined: the post-exp work of unit i-1 (numerator
chains, normalize, transposes) and the phase-A tail (K slabs 4-7, Q 1-3,
V projection) are interleaved between the score groups of unit i so the
in-order PE queue never parks on a not-yet-exp'd psum.

Host sums the 4 head-group partials per shard, adds b_out, and scatters
the mod-2 interleaved rows back.
"""
import sys

sys.path.insert(0, "/opt/trn_rl_repo")

import numpy as np
import ml_dtypes

S, D, H, DH = 4096, 768, 12, 64
N_CORES = 8
GD = 192          # head-group dims (3 heads)
LT = 16           # local 128-row subtiles per core (2048 q rows)
NK = 32           # key tiles

# Schraudolph constants: code = round(s * SCH_A + SCH_B), bitcast bf16.
# SCH_A = 128*log2(e); SCH_B tuned so the multiplicative sawtooth error is
# zero-mean over uniform fractionals.
SCH_A = 184.66496736282892
SCH_B = 16248.67

_cache = {}
DBG_ALL_ACT = False
DBG_DUMP = False


def _build_program(sc_bufs=3, ypk_bufs=1, misc_bufs=1):
    import concourse.bass as bass
    import concourse.mybir as mybir
    import concourse.tile as tile
    from concourse import bacc

    f32 = mybir.dt.float32
    bf16 = mybir.dt.bfloat16
    i16 = mybir.dt.int16
    fp8 = mybir.dt.float8e4
    DR = mybir.MatmulPerfMode.DoubleRow
    Exp = mybir.ActivationFunctionType.Exp
    mult = mybir.AluOpType.mult
    add = mybir.AluOpType.add

    nc = bacc.Bacc(
        "TRN2",
        target_bir_lowering=False,
        debug=False,
        enable_asserts=False,
        num_devices=N_CORES,
    )

    d_xT = nc.dram_tensor("xt_in", [D, S], bf16, kind="ExternalInput").ap()
    d_xTq = nc.dram_tensor("xtq_in", [D, 2048], bf16, kind="ExternalInput").ap()
    d_wq = nc.dram_tensor("wq_in", [D, GD], bf16, kind="ExternalInput").ap()
    d_wk = nc.dram_tensor("wk_in", [D, GD], bf16, kind="ExternalInput").ap()
    d_wv = nc.dram_tensor("wv_in", [D, GD], bf16, kind="ExternalInput").ap()
    d_bq = nc.dram_tensor("bq_in", [GD, 1], f32, kind="ExternalInput").ap()
    d_bk = nc.dram_tensor("bk_in", [GD, 1], f32, kind="ExternalInput").ap()
    d_bvb = nc.dram_tensor("bvb_in", [128, GD], f32, kind="ExternalInput").ap()
    d_wo = nc.dram_tensor("wo_in", [GD, D], bf16, kind="ExternalInput").ap()
    d_mm = nc.dram_tensor("mm_in", [2, 128, 128], bf16, kind="ExternalInput").ap()
    d_id = nc.dram_tensor("id_in", [128, 128], bf16, kind="ExternalInput").ap()
    d_out = nc.dram_tensor("outt_out", [D, 2048], f32, kind="ExternalOutput").ap()
    d_dbg = {}
    if DBG_DUMP:
        d_dbg["kt0"] = nc.dram_tensor("kt0_out", [128, S], bf16, kind="ExternalOutput").ap()
        d_dbg["kt1"] = nc.dram_tensor("kt1_out", [64, S], bf16, kind="ExternalOutput").ap()
        d_dbg["qt0"] = nc.dram_tensor("qt0_out", [128, 2048], bf16, kind="ExternalOutput").ap()
        d_dbg["qt1"] = nc.dram_tensor("qt1_out", [64, 2048], bf16, kind="ExternalOutput").ap()
        d_dbg["vb"] = nc.dram_tensor("vb_out", [128, 3, NK, 65], bf16, kind="ExternalOutput").ap()
        d_dbg["yt0"] = nc.dram_tensor("yt0_out", [128, 2048], bf16, kind="ExternalOutput").ap()
        d_dbg["yt1"] = nc.dram_tensor("yt1_out", [64, 2048], bf16, kind="ExternalOutput").ap()

    # scores kt groups
    GSZ = 2
    groups = [(k0, min(GSZ, NK - k0)) for k0 in range(0, NK, GSZ)]

    with tile.TileContext(nc) as tc:
        xTr = d_xT.rearrange("(k p) n -> p k n", p=128)     # [128, 6, 4096]
        xTqr = d_xTq.rearrange("(k p) n -> p k n", p=128)   # [128, 6, 2048]
        wqr = d_wq.rearrange("(k p) m -> p k m", p=128)
        wkr = d_wk.rearrange("(k p) m -> p k m", p=128)
        wvr = d_wv.rearrange("(k p) m -> p k m", p=128)
        mmr = d_mm.rearrange("m p n -> p m n")
        with tc.tile_pool(name="const", bufs=1) as cp:
            wq_sb = cp.tile([128, 6, GD], bf16, tag="wq")
            wk_sb = cp.tile([128, 6, GD], bf16, tag="wk")
            wv_sb = cp.tile([128, 6, GD], bf16, tag="wv")
            wo0 = cp.tile([128, D], bf16, tag="wo0")
            wo1 = cp.tile([64, D], bf16, tag="wo1")
            bq0 = cp.tile([128, 1], f32, tag="bq0")
            bq1 = cp.tile([64, 1], f32, tag="bq1")
            bk0 = cp.tile([128, 1], f32, tag="bk0")
            bk1 = cp.tile([64, 1], f32, tag="bk1")
            bvb = cp.tile([128, GD], f32, tag="bvb")
            mm_sb = cp.tile([128, 2, 128], bf16, tag="mm")
            id_sb = cp.tile([128, 128], bf16, tag="ident")
            ones = cp.tile([128, 1], bf16, tag="ones")
            zc = cp.tile([128, 65], bf16, tag="zc")
            KT0 = cp.tile([128, S], bf16, tag="KT0")
            KT1 = cp.tile([64, S], bf16, tag="KT1")
            QT0 = cp.tile([128, 2048], bf16, tag="QT0")
            QT1 = cp.tile([64, 2048], bf16, tag="QT1")
            Vb = cp.tile([128, 3, NK, 65], bf16, tag="Vb")
            yT0 = cp.tile([128, 2048], bf16, tag="yT0")
            yT1 = cp.tile([64, 2048], bf16, tag="yT1")
            pk0 = cp.tile([128, 16, 128], bf16, tag="pk0")  # packed ysb h0|h1
            KT8 = cp.tile([32, 3, 2, S], fp8, tag="KT8")
            QT8 = cp.tile([32, 3, 2, 2048], fp8, tag="QT8")

            nc.sync.dma_start(out=wq_sb[:], in_=wqr[:])
            nc.sync.dma_start(out=bq0[:], in_=d_bq[0:128, :])
            nc.sync.dma_start(out=bq1[:], in_=d_bq[128:GD, :])
            nc.sync.dma_start(out=wk_sb[:], in_=wkr[:])
            nc.sync.dma_start(out=bk0[:], in_=d_bk[0:128, :])
            nc.sync.dma_start(out=bk1[:], in_=d_bk[128:GD, :])
            nc.vector.memset(ones[:], 1.0)
            nc.vector.memset(zc[:], 0.0)
            nc.vector.memset(Vb[:, :, :, 64:65], 1.0)

            def late_const_dmas():
                nc.sync.dma_start(out=wv_sb[:], in_=wvr[:])
                nc.sync.dma_start(out=mm_sb[:], in_=mmr[:])
                nc.sync.dma_start(out=id_sb[:], in_=d_id[:, :])
                nc.sync.dma_start(out=bvb[:], in_=d_bvb[:, :])
                nc.sync.dma_start(out=wo0[:], in_=d_wo[0:128, :])
                nc.sync.dma_start(out=wo1[:], in_=d_wo[128:GD, :])

            with (
                tc.tile_pool(name="xsl", bufs=4) as xp,
                tc.tile_pool(name="psA", bufs=sc_bufs, space="PSUM") as pa,
                tc.tile_pool(name="psK", bufs=ypk_bufs, space="PSUM") as pyk,
                tc.tile_pool(name="psM", bufs=misc_bufs, space="PSUM") as pm,
                tc.tile_pool(name="epool", bufs=2) as ep,
                tc.tile_pool(name="small", bufs=3) as sp,
                tc.tile_pool(name="ocp", bufs=2) as op_,
            ):
                def qproj(qb):
                    xq = xp.tile([128, 6, 512], bf16, tag="xq", name=f"xq{qb}")
                    nc.sync.dma_start(
                        out=xq[:], in_=xTqr[:, :, qb * 512:(qb + 1) * 512]
                    )
                    for i, (msz, off, QT_t, bq_t) in enumerate(
                        ((128, 0, QT0, bq0), (64, 128, QT1, bq1))
                    ):
                        ps = pm.tile([msz, 512], f32, tag="pm", name=f"qps{qb}_{i}")
                        for k in range(6):
                            nc.tensor.matmul(
                                ps[:],
                                wq_sb[:, k, off:off + msz],
                                xq[:, k, :],
                                start=(k == 0),
                                stop=(k == 5),
                            )
                        nc.vector.tensor_scalar_add(
                            QT_t[:, qb * 512:(qb + 1) * 512], ps[:], bq_t[:]
                        )

                def kslab_part(nb, i, xs_box):
                    if i == 0:
                        xs = xp.tile([128, 6, 512], bf16, tag="xq", name=f"xs{nb}")
                        nc.sync.dma_start(
                            out=xs[:], in_=xTr[:, :, nb * 512:(nb + 1) * 512]
                        )
                        xs_box[nb] = xs
                    xs = xs_box[nb]
                    msz, off, KT_t, bk_t = (
                        (128, 0, KT0, bk0) if i == 0 else (64, 128, KT1, bk1)
                    )
                    ps = pm.tile([msz, 512], f32, tag="pm", name=f"kps{nb}_{i}")
                    for k in range(6):
                        nc.tensor.matmul(
                            ps[:],
                            wk_sb[:, k, off:off + msz],
                            xs[:, k, :],
                            start=(k == 0),
                            stop=(k == 5),
                        )
                    nc.vector.tensor_scalar_add(
                        KT_t[:, nb * 512:(nb + 1) * 512], ps[:], bk_t[:]
                    )

                xs_box = {}

                def kslab(nb):
                    kslab_part(nb, 0, xs_box)
                    kslab_part(nb, 1, xs_box)

                def vproj_part(nb, ms, xv_box):
                    if ms == 0:
                        xv = xp.tile([128, 6, 512], bf16, tag="xv",
                                     name=f"xv{nb}", bufs=2)
                        nc.sync.dma_start(
                            out=xv[:], in_=xTr[:, :, nb * 512:(nb + 1) * 512]
                        )
                        xv_box[nb] = xv
                    xv = xv_box[nb]
                    for ms in (ms,):
                        kt = nb * 4 + ms
                        ps = pm.tile([128, GD], f32, tag="pm", name=f"vps{nb}_{ms}")
                        for k in range(6):
                            nc.tensor.matmul(
                                ps[:],
                                xv[:, k, ms * 128:(ms + 1) * 128],
                                wv_sb[:, k, :],
                                start=(k == 0),
                                stop=(k == 5),
                            )
                        nc.vector.tensor_tensor(
                            Vb[:, :, kt, 0:64],
                            ps[:].rearrange("p (h d) -> p h d", h=3),
                            bvb[:].rearrange("p (h d) -> p h d", h=3),
                            add,
                        )

                KSC, QSC = 1.0 / 2.8284271247461903, 2.8284271247461903

                def conv_k(h, i, half):
                    srcp = KT0[64 * h + 32 * i:64 * h + 32 * i + 32, :] if h < 2 \
                        else KT1[32 * i:32 * i + 32, :]
                    sl = slice(half * 2048, (half + 1) * 2048)
                    nc.gpsimd.tensor_scalar_mul(
                        KT8[:, h, i, sl], srcp[:, sl], KSC
                    )

                def conv_q(h, i):
                    srcp = QT0[64 * h + 32 * i:64 * h + 32 * i + 32, :] if h < 2 \
                        else QT1[32 * i:32 * i + 32, :]
                    nc.gpsimd.tensor_scalar_mul(QT8[:, h, i, :], srcp, QSC)

                conv_chunks = (
                    [(lambda h=h, i=i, half=half: conv_k(h, i, half))
                     for h in range(3) for i in range(2) for half in range(2)]
                    + [(lambda h=h, i=i: conv_q(h, i))
                       for h in range(3) for i in range(2)]
                )

                xv_box = {}

                def vproj_chunks(nb):
                    return [
                        (lambda ms=ms: vproj_part(nb, ms, xv_box))
                        for ms in range(4)
                    ]

                # ---- phase A head: Q(0) and K slabs 0-3 emitted directly ----
                qproj(0)
                for nb in range(4):
                    kslab(nb)

                # phase A tail: K slabs 4-7 woven into the first unit (its
                # upper groups need them progressively); Q/V spread over the
                # following positions, early enough for each consumer.
                late_const_dmas()
                phase_a_assign = {
                    0: [lambda nb=nb, i=i: kslab_part(nb, i, xs_box)
                        for nb in range(4, 8) for i in range(2)],
                    1: ([lambda: qproj(3)] + vproj_chunks(0) + vproj_chunks(1)
                        + vproj_chunks(2) + vproj_chunks(3) + conv_chunks),
                    2: (vproj_chunks(4) + vproj_chunks(5) + vproj_chunks(6)
                        + vproj_chunks(7) + [lambda: qproj(1)]),
                    3: [lambda: qproj(2)],
                }

                def scores_emitters(qb, h, first=False, dr_ok=False):
                    """16 closures, one per 2-kt score group (matmuls + exp)."""
                    bnd = 8 * qb + 8
                    if h < 2:
                        KTh = KT0[64 * h:64 * (h + 1), :]
                        QTh = QT0[64 * h:64 * (h + 1), :]
                    else:
                        KTh = KT1[0:64, :]
                        QTh = QT1[0:64, :]
                    E = ep.tile([128, NK, 512], bf16, tag="E", name=f"E{qb}_{h}")
                    Ei = E[:].bitcast(i16)

                    def emit_group(k0, gsz):
                        ps = pa.tile(
                            [128, GSZ * 512], f32, tag="psA", name=f"sc{qb}_{h}_{k0}"
                        )
                        use_dr = dr_ok and k0 >= bnd
                        for i in range(gsz):
                            kt = k0 + i
                            if use_dr:
                                nc.tensor.matmul(
                                    ps[:, i * 512:(i + 1) * 512],
                                    KT8[:, h, :, kt * 128:(kt + 1) * 128],
                                    QT8[:, h, :, qb * 512:(qb + 1) * 512],
                                    start=True,
                                    stop=True,
                                    perf_mode=DR,
                                )
                            else:
                                nc.tensor.matmul(
                                    ps[:, i * 512:(i + 1) * 512],
                                    KTh[:, kt * 128:(kt + 1) * 128],
                                    QTh[:, qb * 512:(qb + 1) * 512],
                                    start=True,
                                    stop=True,
                                )
                        lo = min(max(bnd - k0, 0), gsz)
                        if DBG_ALL_ACT:
                            lo = gsz
                        if lo > 0:
                            nc.scalar.activation(
                                E[:, k0:k0 + lo, :], ps[:, 0:lo * 512], Exp
                            )
                        if lo < gsz:
                            nc.vector.tensor_scalar(
                                Ei[:, k0 + lo:k0 + gsz, :],
                                ps[:, lo * 512:gsz * 512],
                                SCH_A, SCH_B, mult, add,
                            )

                    # interleave ACT-destined (k0 < bnd) and DVE-destined
                    # groups so both exp engines run concurrently
                    a_list = [g for g in groups if g[0] < bnd]
                    u_list = [g for g in groups if g[0] >= bnd]
                    weave = []
                    na, nu = len(a_list), len(u_list)
                    ia = iu = 0
                    for i in range(na + nu):
                        # pace the shorter list evenly through the longer one
                        if ia * max(nu, 1) <= iu * max(na, 1) and ia < na or iu >= nu:
                            weave.append(a_list[ia]); ia += 1
                        else:
                            weave.append(u_list[iu]); iu += 1
                    if first:
                        weave = groups  # ascending: K slabs arrive in order
                    return E, [
                        (lambda k0=k0, gsz=gsz: emit_group(k0, gsz))
                        for k0, gsz in weave
                    ]

                def post_emitters(qb, h, E):
                    """Consumer chunks for a finished unit: em + 4 chains.
                    All four 65-col chains plus the transpose region pack
                    into one psum tile (1 bank) to relieve bank pressure."""
                    chunks = []
                    em_all = {}
                    ypk = pyk.tile([128, 324], f32, tag="ypk", name=f"ypk{qb}_{h}")

                    def emit_em(tl):
                        t = qb * 4 + tl
                        tsl = slice(tl * 128, (tl + 1) * 128)
                        for j in range(2):
                            em = sp.tile([128, 128], bf16, tag="em",
                                         name=f"em{t}_{h}_{j}", bufs=10)
                            nc.gpsimd.tensor_tensor(
                                em[:], E[:, 2 * t + j, tsl], mm_sb[:, j, :], mult
                            )
                            em_all[(tl, j)] = em

                    def emit_chain_half(tl, half):
                        t = qb * 4 + tl
                        tsl = slice(tl * 128, (tl + 1) * 128)
                        o = tl * 65
                        ypn = ypk[:, o:o + 64]
                        ypd = ypk[:, o + 64:o + 65]
                        ypf = ypk[:, o:o + 65]
                        mid = t  # split fulls at kt=t
                        if half == 0:
                            if t == 0:
                                # zero the 65-col region in one start group
                                nc.tensor.matmul(ypf, E[:, 0, tsl], zc[:],
                                                 start=True, stop=False)
                            for kt in range(mid):
                                nc.tensor.matmul(
                                    ypf, E[:, kt, tsl], Vb[:, h, kt, :],
                                    start=(kt == 0), stop=False,
                                )
                            return
                        for kt in range(mid, 2 * t):
                            nc.tensor.matmul(
                                ypf, E[:, kt, tsl], Vb[:, h, kt, :],
                                start=(kt == 0), stop=False,
                            )
                        for j in range(2):
                            kt = 2 * t + j
                            nc.tensor.matmul(
                                ypn, em_all[(tl, j)][:],
                                Vb[:, h, kt, 0:64],
                                start=False, stop=False,
                            )
                            nc.tensor.matmul(
                                ypd, E[:, kt, tsl], ones[:],
                                start=False,
                                stop=(t == 15 and j == 1),
                            )
                        for kt in range(2 * t + 2, NK):
                            nc.tensor.matmul(
                                ypd, E[:, kt, tsl], ones[:],
                                start=False, stop=(kt == NK - 1),
                            )

                    rc4 = sp.tile([128, 4], f32, tag="rc",
                                  name=f"rc4_{qb}_{h}", bufs=3)

                    def emit_fin(tl):
                        t = qb * 4 + tl
                        o = tl * 65
                        ypn = ypk[:, o:o + 64]
                        if tl == 0:
                            nc.vector.reciprocal(
                                rc4[:], ypk[:, 64:65 + 3 * 65:65]
                            )
                        if h < 2:
                            dst = pk0[:, t, 64 * h:64 * (h + 1)]
                        else:
                            dst = sp.tile([128, 64], bf16, tag="ysb2",
                                          name=f"ysb2{t}", bufs=3)
                        nc.vector.tensor_scalar_mul(dst, ypn, rc4[:, tl:tl + 1])
                        if h == 1:
                            tp = ypk[:, 260:324].bitcast(bf16)
                            nc.tensor.transpose(tp, pk0[:, t, :], id_sb[:])
                            nc.vector.tensor_copy(
                                yT0[:, t * 128:(t + 1) * 128], tp
                            )
                        elif h == 2:
                            tp = ypk[0:64, 260:324].bitcast(bf16)
                            nc.tensor.transpose(tp, dst, id_sb[:])
                            nc.vector.tensor_copy(
                                yT1[:, t * 128:(t + 1) * 128], tp
                            )

                    for tl in range(4):
                        chunks.append(lambda tl=tl: emit_em(tl))
                        chunks.append(lambda tl=tl: emit_chain_half(tl, 0))
                        chunks.append(lambda tl=tl: emit_chain_half(tl, 1))
                    fins = [lambda tl=tl: emit_fin(tl) for tl in range(4)]
                    return chunks, fins

                def wout_emitters(qb):
                    def emit_wout(mt):
                        csl = slice(qb * 512, (qb + 1) * 512)
                        ps = pm.tile([128, 512], f32, tag="pm", name=f"ops{qb}_{mt}")
                        nc.tensor.matmul(
                            ps[:], wo0[:, mt * 128:(mt + 1) * 128], yT0[:, csl],
                            start=True, stop=False,
                        )
                        nc.tensor.matmul(
                            ps[:], wo1[:, mt * 128:(mt + 1) * 128], yT1[:, csl],
                            start=False, stop=True,
                        )
                        oc = op_.tile([128, 512], f32, tag="ocp", name=f"oc{qb}_{mt}")
                        nc.scalar.copy(oc[:], ps[:])
                        nc.sync.dma_start(
                            out=d_out[mt * 128:(mt + 1) * 128, csl], in_=oc[:]
                        )

                    return [lambda mt=mt: emit_wout(mt) for mt in range(6)]

                # ---- software-pipelined emission over unit PAIRS ----
                # Each pair couples a DVE-heavy unit (low qb: mostly
                # Schraudolph groups) with an ACT-heavy one (high qb) and
                # interleaves their score groups, so both exp engines stay
                # fed throughout the pair. Consumer work (chains, fins,
                # wout) trails by one pair via the pending lists.
                pairs = [
                    [(0, 0)],
                    [(0, 1), (3, 0)],
                    [(0, 2), (3, 1)],
                    [(1, 0), (3, 2)],
                    [(1, 1), (2, 0)],
                    [(1, 2), (2, 1)],
                    [(2, 2)],
                ]
                npos = len(pairs)
                sched = {p: list(phase_a_assign.get(p, [])) for p in range(npos + 2)}
                for pos, members in enumerate(pairs):
                    scs = []
                    tail = []
                    for qb, h in members:
                        E, sc = scores_emitters(qb, h, first=(pos == 0),
                                                dr_ok=(pos >= 2))
                        chains, fins = post_emitters(qb, h, E)
                        tail.extend(chains)
                        tail.extend(fins)
                        if h == 2:
                            tail.extend(wout_emitters(qb))
                        scs.append(sc)
                    # interleave members' groups; second member offset by 4
                    if len(scs) == 1:
                        weave = scs[0]
                    else:
                        a, b = scs
                        weave = list(a[:4])
                        ra, rb = a[4:], b
                        for i in range(max(len(ra), len(rb))):
                            if i < len(ra):
                                weave.append(ra[i])
                            if i < len(rb):
                                weave.append(rb[i])
                    sched[min(pos + 1, npos + 1)].extend(tail)
                    pending = sched[pos]
                    done = 0
                    n = len(pending)
                    ng = len(weave)
                    for g, cb in enumerate(weave):
                        cb()
                        want = (n * (g + 1) + ng - 1) // ng
                        while done < min(want, n):
                            pending[done]()
                            done += 1
                    while done < n:
                        pending[done]()
                        done += 1
                for p in range(npos, npos + 2):
                    for cb in sched[p]:
                        cb()
                if DBG_DUMP:
                    nc.sync.dma_start(out=d_dbg["kt0"][:, :], in_=KT0[:])
                    nc.sync.dma_start(out=d_dbg["kt1"][:, :], in_=KT1[:])
                    nc.sync.dma_start(out=d_dbg["qt0"][:, :], in_=QT0[:])
                    nc.sync.dma_start(out=d_dbg["qt1"][:, :], in_=QT1[:])
                    nc.sync.dma_start(out=d_dbg["vb"][:, :, :, :], in_=Vb[:])
                    nc.sync.dma_start(out=d_dbg["yt0"][:, :], in_=yT0[:])
                    nc.sync.dma_start(out=d_dbg["yt1"][:, :], in_=yT1[:])

    nc.compile()
    return nc


def _get_program(sc_bufs=3, ypk_bufs=1, misc_bufs=1):
    key = ("nc", sc_bufs, ypk_bufs, misc_bufs)
    if key not in _cache:
        _cache[key] = _build_program(sc_bufs, ypk_bufs, misc_bufs)
    return _cache[key]


def _qcols(s):
    idx = []
    for t in range(LT):
        g = 2 * t + s
        idx.extend(range(g * 128, g * 128 + 128))
    return np.array(idx)


def _masks(s):
    # em = E[k_part, q_free] * M_j for band tile kt = 2t + j, q-tile g = 2t+s
    tri = (np.arange(128)[:, None] <= np.arange(128)[None, :]).astype(np.float32)
    M = np.zeros((2, 128, 128), np.float32)
    if s == 0:
        M[0] = tri          # kt == g
        M[1] = 0.0          # kt == g+1 (future)
    else:
        M[0] = 1.0          # kt == g-1 (past)
        M[1] = tri          # kt == g
    return M


def shard_inputs(x, W_qkv, b_qkv, W_out, b_out):
    """Build the 8 per-core input maps."""
    bf = ml_dtypes.bfloat16
    xT = np.ascontiguousarray(x[0].T.astype(np.float32)).astype(bf)  # [D, S]
    ident = np.eye(128, dtype=np.float32).astype(bf)
    in_maps = []
    per_s = {}
    for s in (0, 1):
        cols = _qcols(s)
        per_s[s] = (
            np.ascontiguousarray(xT[:, cols]),
            _masks(s).astype(bf),
        )
    for c in range(N_CORES):
        hg, s = c // 2, c % 2
        hsl = slice(GD * hg, GD * (hg + 1))
        xTq, M = per_s[s]
        wq = np.ascontiguousarray((W_qkv[0:768][hsl] / 8.0).T.astype(np.float32)).astype(bf)
        wk = np.ascontiguousarray(W_qkv[768:1536][hsl].T.astype(np.float32)).astype(bf)
        wv = np.ascontiguousarray(W_qkv[1536:2304][hsl].T.astype(np.float32)).astype(bf)
        bq = (b_qkv[0:768][hsl] / 8.0).astype(np.float32).reshape(GD, 1)
        bk = b_qkv[768:1536][hsl].astype(np.float32).reshape(GD, 1)
        bv = b_qkv[1536:2304][hsl].astype(np.float32)
        bvb = np.ascontiguousarray(np.broadcast_to(bv[None, :], (128, GD)))
        wo = np.ascontiguousarray(W_out[:, hsl].T.astype(np.float32)).astype(bf)
        in_maps.append(
            {
                "xt_in": xT,
                "xtq_in": xTq,
                "wq_in": wq,
                "wk_in": wk,
                "wv_in": wv,
                "bq_in": bq,
                "bk_in": bk,
                "bvb_in": bvb,
                "wo_in": wo,
                "mm_in": M,
                "id_in": ident,
            }
        )
    return in_maps


def gather_output(results, b_out):
    out = np.zeros((S, D), np.float32)
    for s in (0, 1):
        acc = np.zeros((2048, D), np.float32)
        for hg in range(4):
            c = hg * 2 + s
            acc += results[c]["outt_out"].T
        out[_qcols(s)] = acc + b_out[None, :].astype(np.float32)
    return out.reshape(1, S, D)


def kernel(x, W_qkv, b_qkv, W_out, b_out):
    from concourse.bass_utils import run_bass_kernel_spmd

    x = np.asarray(x)
    W_qkv = np.asarray(W_qkv)
    b_qkv = np.asarray(b_qkv)
    W_out = np.asarray(W_out)
    b_out = np.asarray(b_out)
    nc = _get_program()
    in_maps = shard_inputs(x, W_qkv, b_qkv, W_out, b_out)
    res = run_bass_kernel_spmd(nc, in_maps, list(range(N_CORES)))
    return gather_output(res.results, b_out)
